# revision 8
# baseline (speedup 1.0000x reference)
"""AttLIF Trainium2 kernel (8-core data-parallel SPMD).

Reference computation (per batch shard):
  x = data @ W.T + b                       # Linear [B,T,I]->[B,T,H]
  s = mean_h(x); a = sigmoid(relu(s@w1.T+b1)@w2.T+b2)   # TA gate [B,T]
  x = x * a[:, :, None]
  LIF over T: u = a*u + x_t; sp = (u>=VTH); u *= (1-sp) # hard reset

Strategy (v2 - time-chunked for LIF/matmul overlap):
  - Shard B=128 over 8 cores (16 each); W replicated, fp16 single pass
    (measured spike L2 err ~1.6% vs the 2% gate; fp32 floor 0.05%).
  - Tokens laid out globally t-major (token = t*16 + b) so the matmul can
    be chunked along TIME: chunks t=[0,24), [24,48), [48,64) with all 16
    local batches (N = 384/384/256 moving dim, wide enough that
    LDWEIGHTS stays hidden under the matmul stream).
  - W is loaded ONCE into a resident SBUF tile (8.4 MB fp16) in hc order
    during chunk 0's sweep; later chunks re-read it from SBUF for free.
  - After chunk c's 16x16 (hc,ic) matmul sweep + PSUM drains, its LIF
    steps run on DVE while the PE does chunk c+1 -> only the final
    16-step chain (~14 us) is exposed after the last matmul.
  - s computed per-chunk as wbar @ dat (wbar = col-mean of W) so the TA
    MLP finishes early; gate applied in the PSUM drain (one
    scalar_tensor_tensor per (chunk,hc)).
  - Spikes: Sign(u-VTH) in {-1,0,1} on the Scalar engine per step pair,
    written fp8 into per-chunk slabs, DMA'd out in 12-step (final chunk:
    4-step) slices; host clamps -1 -> 0.
All host-side work is layout/weight preprocessing only (transposes,
precision casts, column means of W); every data-dependent FLOP runs on
device.
"""

import functools
import numpy as np

ALPHA = 0.3
VTH = 0.3
B, T, I, H = 128, 64, 2048, 2048
NCORES = 8
BL = B // NCORES          # local batch = 16
TOK = BL * T              # 1024 tokens per core (token = t*BL + b)
IC = I // 128             # 16 contraction chunks
HC = H // 128             # 16 hidden chunks
CHUNKS = [(0, 24), (24, 48), (48, 64)]   # t-ranges of the matmul chunks
DTMAX = 24


def _dts():
    import ml_dtypes
    return np.float16, ml_dtypes.float8_e4m3


@functools.cache
def _build():
    import sys
    if "/opt/trn_rl_repo" not in sys.path:
        sys.path.insert(0, "/opt/trn_rl_repo")
    from contextlib import ExitStack
    from concourse import bacc, mybir, tile

    f32 = mybir.dt.float32
    f16 = mybir.dt.float16
    f8 = mybir.dt.float8e4
    Alu = mybir.AluOpType
    Act = mybir.ActivationFunctionType

    nc = bacc.Bacc("TRN2", target_bir_lowering=False, debug=False)

    dat_d = nc.dram_tensor("dat", [I, TOK], f16, kind="ExternalInput")
    wt_d = nc.dram_tensor("wt", [I, H], f16, kind="ExternalInput")
    bias_d = nc.dram_tensor("bias", [128, HC], f32, kind="ExternalInput")
    wbar_d = nc.dram_tensor("wbar", [128, IC], f16, kind="ExternalInput")
    bbar_d = nc.dram_tensor("bbar", [1, 1], f32, kind="ExternalInput")
    w1r_d = nc.dram_tensor("w1r", [BL, 4, T], f32, kind="ExternalInput")
    b1r_d = nc.dram_tensor("b1r", [BL, 4], f32, kind="ExternalInput")
    w2r_d = nc.dram_tensor("w2r", [BL, T, 4], f32, kind="ExternalInput")
    b2r_d = nc.dram_tensor("b2r", [BL, T], f32, kind="ExternalInput")
    spk_d = nc.dram_tensor("spk", [128, T, HC, BL], f8, kind="ExternalOutput")

    s_dram = nc.dram_tensor("s_scratch", [TOK], f32)
    a_dram = nc.dram_tensor("a_scratch", [T, BL], f32)

    with ExitStack() as ctx:
        tc = ctx.enter_context(tile.TileContext(nc))
        const = ctx.enter_context(tc.tile_pool(name="const", bufs=1))
        xpool = ctx.enter_context(tc.tile_pool(name="xpool", bufs=2))
        spool = ctx.enter_context(tc.tile_pool(name="spool", bufs=2))
        upool = ctx.enter_context(tc.tile_pool(name="upool", bufs=1))
        psum = ctx.enter_context(tc.tile_pool(name="psum", bufs=7, space="PSUM"))
        psum_s = ctx.enter_context(tc.tile_pool(name="psum_s", bufs=1, space="PSUM"))

        # ---- persistent loads (data on the ACT HWDGE ring, W on Sync) ----
        dat_sb = const.tile([128, IC, TOK], f16, tag="dat")
        datv = dat_d.ap().rearrange("(ic p) tok -> p ic tok", p=128)

        def emit_data_chunk(ci):
            t0, t1 = CHUNKS[ci]
            sl = slice(t0 * BL, t1 * BL)
            for icc in range(0, IC, 8):
                nc.scalar.dma_start(
                    out=dat_sb[:, icc : icc + 8, sl], in_=datv[:, icc : icc + 8, sl]
                )

        # resident weights, loaded once in hc order; separate full tiles per
        # 256-col piece so each DMA is a whole-tile write (clean deps)
        wp = [
            const.tile([128, IC, 256], f16, tag=f"wp{k}", name=f"wp{k}")
            for k in range(8)
        ]

        def emit_w_pair(k):
            h0 = k * 256
            nc.sync.dma_start(
                out=wp[k],
                in_=wt_d[:, h0 : h0 + 256].rearrange("(ic p) h -> p ic h", p=128),
            )

        emit_data_chunk(0)
        emit_w_pair(0)
        wbar_sb = const.tile([128, IC], f16, tag="wbar")
        nc.sync.dma_start(out=wbar_sb, in_=wbar_d.ap())
        bias_sb = const.tile([128, HC], f32, tag="bias")
        nc.sync.dma_start(out=bias_sb, in_=bias_d.ap())
        bbar_sb = const.tile([1, 1], f32, tag="bbar")
        nc.sync.dma_start(out=bbar_sb, in_=bbar_d.ap())
        w1r_sb = const.tile([BL, 4, T], f32, tag="w1r")
        nc.sync.dma_start(out=w1r_sb, in_=w1r_d.ap())
        b1r_sb = const.tile([BL, 4], f32, tag="b1r")
        nc.sync.dma_start(out=b1r_sb, in_=b1r_d.ap())
        w2r_sb = const.tile([BL, T, 4], f32, tag="w2r")
        nc.sync.dma_start(out=w2r_sb, in_=w2r_d.ap())
        b2r_sb = const.tile([BL, T], f32, tag="b2r")
        nc.sync.dma_start(out=b2r_sb, in_=b2r_d.ap())
        nvth_sb = const.tile([128, 1], f32, tag="nvth")
        nc.vector.memset(nvth_sb, -VTH)
        emit_data_chunk(1)
        emit_w_pair(1)
        emit_data_chunk(2)
        for k in range(2, 8):
            emit_w_pair(k)

        # ---- gate: squeeze s per chunk, TA MLP once, broadcast ----
        s_sb = const.tile([1, TOK], f32, tag="s")
        a_rep = const.tile([128, T, BL], f32, tag="a_rep")

        def emit_squeeze(ci):
            t0, t1 = CHUNKS[ci]
            n = (t1 - t0) * BL
            ps = psum_s.tile([1, 384], f32, tag="ps_s", name=f"ps_s{ci}")
            for ic in range(IC):
                nc.tensor.matmul(
                    ps[:, :n],
                    lhsT=wbar_sb[:, ic : ic + 1],
                    rhs=dat_sb[:, ic, t0 * BL : t1 * BL],
                    start=(ic == 0),
                    stop=(ic == IC - 1),
                )
            nc.vector.tensor_scalar(
                out=s_sb[:, t0 * BL : t1 * BL], in0=ps[:, :n],
                scalar1=bbar_sb, scalar2=None, op0=Alu.add,
            )

        def emit_gate_mlp():
            # bounce through DRAM to re-partition [1,1024] -> [16,64]
            nc.scalar.dma_start(out=s_dram.ap(), in_=s_sb)
            sT_sb = const.tile([BL, T], f32, tag="sT")
            nc.scalar.dma_start(
                out=sT_sb, in_=s_dram.ap().rearrange("(t b) -> b t", b=BL)
            )
            h1_sb = const.tile([BL, 4], f32, tag="h1")
            tmp_sb = const.tile([BL, T], f32, tag="ta_tmp")
            for r in range(4):
                nc.vector.tensor_tensor(
                    out=tmp_sb, in0=sT_sb, in1=w1r_sb[:, r : r + 1, :], op=Alu.mult
                )
                nc.vector.tensor_reduce(
                    out=h1_sb[:, r : r + 1], in_=tmp_sb,
                    axis=mybir.AxisListType.X, op=Alu.add,
                )
            nc.vector.tensor_tensor(out=h1_sb, in0=h1_sb, in1=b1r_sb, op=Alu.add)
            h1c_sb = const.tile([BL, 4], f32, tag="h1c")
            nc.scalar.activation(out=h1c_sb, in_=h1_sb, func=Act.Relu)
            acc = [
                const.tile([BL, T], f32, tag=f"acc{r}", name=f"acc{r}")
                for r in range(4)
            ]
            nc.vector.scalar_tensor_tensor(
                out=acc[0], in0=w2r_sb[:, :, 0:1], scalar=h1c_sb[:, 0:1],
                in1=b2r_sb, op0=Alu.mult, op1=Alu.add,
            )
            for r in range(1, 4):
                nc.vector.scalar_tensor_tensor(
                    out=acc[r], in0=w2r_sb[:, :, r : r + 1], scalar=h1c_sb[:, r : r + 1],
                    in1=acc[r - 1], op0=Alu.mult, op1=Alu.add,
                )
            # sigmoid into rows 0:16 of a zeroed [32, T] pad tile, 32x32
            # block-transposes -> [T, 16], bounce via DRAM to broadcast
            a16p_sb = const.tile([32, T], f32, tag="a16p")
            aTp_sb = const.tile([T, 32], f32, tag="aTp")
            nc.vector.memset(a16p_sb, 0.0)
            nc.scalar.activation(out=a16p_sb[:BL, :], in_=acc[3], func=Act.Sigmoid)
            for blk in range(2):
                nc.vector.transpose(
                    out=aTp_sb[32 * blk : 32 * blk + 32, :],
                    in_=a16p_sb[:, 32 * blk : 32 * blk + 32],
                )
            nc.scalar.dma_start(out=a_dram.ap(), in_=aTp_sb[:, :BL])
            nc.scalar.dma_start(
                out=a_rep,
                in_=a_dram.ap().unsqueeze(0).to_broadcast((128, T, BL)),
            )

        # ---- LIF state ----
        u_a = upool.tile([128, HC, BL], f32, tag="u_a")
        ubb = [
            upool.tile([128, 2, HC, BL], f32, tag=f"ubb{i}", name=f"ubb{i}")
            for i in range(4)
        ]
        nc.vector.memset(u_a, 0.0)

        def emit_mm_only(ci, hc):
            t0, t1 = CHUNKS[ci]
            n = (t1 - t0) * BL
            ps = psum.tile([128, 384], f32, tag="ps_mm", name=f"ps_{ci}_{hc}")
            for ic in range(IC):
                nc.tensor.matmul(
                    ps[:, :n],
                    lhsT=wp[hc // 2][:, ic, (hc % 2) * 128 : (hc % 2) * 128 + 128],
                    rhs=dat_sb[:, ic, t0 * BL : t1 * BL],
                    start=(ic == 0),
                    stop=(ic == IC - 1),
                )
            return ps

        def emit_drain(ci, hc, ps, x_sb):
            t0, t1 = CHUNKS[ci]
            n = (t1 - t0) * BL
            # drain: x = (ps + bias) * a  (both APs (t,b)-ordered, contiguous)
            nc.vector.scalar_tensor_tensor(
                out=x_sb[:, : t1 - t0, hc : hc + 1, :],
                in0=ps[:, :n],
                scalar=bias_sb[:, hc : hc + 1],
                in1=a_rep[:, t0:t1, :],
                op0=Alu.add, op1=Alu.mult,
            )

        def emit_lif(ci, x_sb, spk_sb):
            t0, t1 = CHUNKS[ci]
            for t in range(t0, t1):
                x_t = x_sb[:, t - t0, :, :]
                u_b = ubb[(t // 2) % 4][:, t % 2]
                nc.vector.scalar_tensor_tensor(
                    out=u_b, in0=u_a, scalar=ALPHA, in1=x_t,
                    op0=Alu.mult, op1=Alu.add,
                )
                if t % 2 == 1:
                    pair = ubb[(t // 2) % 4][:, :]
                    # Sign(u - VTH) in {-1, 0, 1}; host clamps -1 -> 0
                    nc.scalar.activation(
                        out=spk_sb[:, t - t0 - 1 : t - t0 + 1, :, :],
                        in_=pair, func=Act.Sign, bias=nvth_sb,
                    )
                if t + 1 < T:
                    # final reset is dead work: u never read again
                    nc.vector.scalar_tensor_tensor(
                        out=u_a, in0=u_b, scalar=VTH, in1=u_b,
                        op0=Alu.is_lt, op1=Alu.mult,
                    )
                if ci == len(CHUNKS) - 1:
                    if t % 4 == 3:
                        # final chunk: four 4-step stores on the idle scalar
                        # HWDGE ring so the critical last transfer is small
                        q0 = (t - t0) // 4 * 4
                        nc.scalar.dma_start(
                            out=spk_d.ap()[:, t0 + q0 : t0 + q0 + 4],
                            in_=spk_sb[:, q0 : q0 + 4, :, :],
                        )
                elif (t - t0) % 12 == 11:
                    q0 = (t - t0) // 12 * 12
                    nc.gpsimd.dma_start(
                        out=spk_d.ap()[:, t0 + q0 : t0 + q0 + 12],
                        in_=spk_sb[:, q0 : q0 + 12, :, :],
                    )

        # ---- main: per chunk matmul sweep (+early squeeze), drains, LIF ----
        # Chunk 0 ordering: the TA gate must be EMITTED before any drain
        # (Tile is sequential - a drain emitted before the a_rep write would
        # legitimately read stale garbage). Run the first 7 matmul chains
        # drain-less (holding their PSUM banks) with the squeeze matmuls
        # interleaved, emit the gate MLP + broadcast, then the 7 deferred
        # drains - the PE never stalls and the gate is ready in time.
        NDEFER = 7
        for ci in range(len(CHUNKS)):
            t0, t1 = CHUNKS[ci]
            x_sb = xpool.tile([128, DTMAX, HC, BL], f32, tag="x", name=f"x{ci}")
            spk_sb = spool.tile([128, DTMAX, HC, BL], f8, tag="spk", name=f"spk{ci}")
            if ci == 0:
                pend = []
                for hc in range(NDEFER):
                    pend.append(emit_mm_only(0, hc))
                    if hc < 3:
                        emit_squeeze(hc)
                    elif hc == 3:
                        emit_gate_mlp()
                for hc in range(NDEFER):
                    emit_drain(0, hc, pend[hc], x_sb)
                for hc in range(NDEFER, HC):
                    emit_drain(0, hc, emit_mm_only(0, hc), x_sb)
            else:
                for hc in range(HC):
                    emit_drain(ci, hc, emit_mm_only(ci, hc), x_sb)
            emit_lif(ci, x_sb, spk_sb)

    nc.compile()
    return nc


def _host_prep(data, W, b, w1, b1, w2, b2):
    f16, f8 = _dts()
    data = np.ascontiguousarray(data, dtype=np.float32)
    W = np.ascontiguousarray(W, dtype=np.float32)

    Wh = W.astype(f16)
    wt = np.ascontiguousarray(Wh.T)                     # [I, H] fp16
    bias = np.ascontiguousarray(b.reshape(HC, 128).T, dtype=np.float32)
    wbar = W.mean(axis=0, dtype=np.float64).astype(np.float32)  # [I]
    wbar_t = np.ascontiguousarray(wbar.reshape(IC, 128).T).astype(f16)
    bbar = np.array([[b.mean(dtype=np.float64)]], dtype=np.float32)
    w1r = np.ascontiguousarray(np.broadcast_to(w1[None], (BL, 4, T)), dtype=np.float32)
    b1r = np.ascontiguousarray(np.broadcast_to(b1[None], (BL, 4)), dtype=np.float32)
    w2r = np.ascontiguousarray(np.broadcast_to(w2[None], (BL, T, 4)), dtype=np.float32)
    b2r = np.ascontiguousarray(np.broadcast_to(b2[None], (BL, T)), dtype=np.float32)

    in_maps = []
    for c in range(NCORES):
        # tokens globally t-major: token = t*BL + b
        dc = np.ascontiguousarray(
            data[c * BL : (c + 1) * BL]
            .transpose(1, 0, 2)          # [T, BL, I]
            .reshape(TOK, I)
            .T
        )                                               # [I, TOK] fp32
        dh = dc.astype(f16)
        in_maps.append({
            "dat": dh, "wt": wt,
            "bias": bias, "wbar": wbar_t, "bbar": bbar,
            "w1r": w1r, "b1r": b1r, "w2r": w2r, "b2r": b2r,
        })
    return in_maps


def _gather(results):
    outs = []
    for c in range(NCORES):
        # spikes are Sign(u - VTH) in {-1, 0, 1}; clamp negatives to 0
        spk = np.maximum(results[c]["spk"].astype(np.float32), 0.0)
        outs.append(                                # [128, T, HC, BL]
            np.ascontiguousarray(np.transpose(spk, (3, 1, 2, 0))).reshape(BL, T, H)
        )
    return np.concatenate(outs, axis=0)


def kernel(data, W, b, w1, b1, w2, b2):
    import sys
    if "/opt/trn_rl_repo" not in sys.path:
        sys.path.insert(0, "/opt/trn_rl_repo")
    from concourse.bass_utils import run_bass_kernel_spmd

    nc = _build()
    in_maps = _host_prep(data, W, b, w1, b1, w2, b2)
    res = run_bass_kernel_spmd(nc, in_maps, list(range(NCORES)))
    return _gather(res.results).astype(np.float32)
